# revision 29
# baseline (speedup 1.0000x reference)
"""MultiHeadedAttention Trainium2 kernel.

Problem: B=2, T=2048, D=1024, H=16 heads (DK=64), fp32 in/out, padding mask
on keys. out = softmax(mask(QWq (KWk)^T / 8)) @ (VWv) @ Wo^T + biases.

Sharding (8 cores): core c -> batch b = c//4, head group g = c%4 (4 heads,
256 projection columns). Each core computes its heads' attention and a
partial output projection; host sums the 4 partials per batch (+ bo).

Device layout strategy (per core):
  - activations uploaded TRANSPOSED as bf16: xT [1024, 2048] = x.T so the
    D-contraction sits on partitions for every projection matmul.
  - qT/kT computed as [256, 2048] (transposed projections): scores^T[key,q]
    = kT_h.T-chunk (stationary) x qT_h (moving), contraction dk=64.
  - attention units (q-half x head-pair) are software-pipelined: unit u's
    scores/exp interleave with unit u-1's V matmuls (whose exp inputs are a
    full unit old, so the PE queue never blocks on ScalarE); v-projection
    rides in unit 0's back half, qh=0's output projection in the last unit.
  - exp on ScalarE straight out of PSUM with fused scale (1/8) and
    per-partition mask bias (-30000 on padded keys), bf16 out.
  - attn@V in transposed orientation: out2[dk+1, q] with v augmented by a
    ones column -> row 64 = softmax denominator for free.
  - normalize: reciprocal_approx_fast on the denom row, gpsimd
    partition_broadcast, DVE multiply -> x_headsT [256, 2048] bf16 which is
    exactly the stationary operand for the output projection.
  - out_partial[t, :] = x_headsT-chunk (stationary) x WoT (moving), fp32.
"""

import numpy as np
import ml_dtypes

import concourse.bass as bass
import concourse.bacc as bacc
import concourse.tile as tile
from concourse import mybir
from concourse.bass_utils import run_bass_kernel_spmd

B, T, D, H = 2, 2048, 1024, 16
DK = D // H  # 64
GH = 4       # heads per core
GC = GH * DK  # 256 proj columns per core
NCORES = 8
KC = T // 128   # 16 key chunks
DCH = D // 128  # 8 contraction chunks
F32 = mybir.dt.float32
BF16 = mybir.dt.bfloat16

MASK_NEG = -30000.0


def build_program(with_bv: bool, debug_taps: bool = False):
    nc = bacc.Bacc("TRN2")

    # ---- DRAM parameters (per-core shapes) ----
    xq_d = nc.declare_dram_parameter("xq", [DCH, 128, T], BF16, isOutput=False)
    xk_d = nc.declare_dram_parameter("xk", [DCH, 128, T], BF16, isOutput=False)
    xv_d = nc.declare_dram_parameter("xv", [DCH, 128, T], BF16, isOutput=False)
    wq_d = nc.declare_dram_parameter("wq", [128, DCH, GC], BF16, isOutput=False)
    wk_d = nc.declare_dram_parameter("wk", [128, DCH, GC], BF16, isOutput=False)
    wv_d = nc.declare_dram_parameter("wv", [128, DCH, GC], BF16, isOutput=False)
    wo_d = nc.declare_dram_parameter("wo", [128, 2, D], BF16, isOutput=False)
    mask_d = nc.declare_dram_parameter("maskb", [128, KC], F32, isOutput=False)
    bq_d = nc.declare_dram_parameter("bq", [128, 2], F32, isOutput=False)
    bk_d = nc.declare_dram_parameter("bk", [128, 2], F32, isOutput=False)
    bv_d = nc.declare_dram_parameter("bv", [64, GH], F32, isOutput=False)
    out_d = nc.declare_dram_parameter("out", [T, D], F32, isOutput=True)
    if debug_taps:
        tap_qT = nc.declare_dram_parameter("tap_qT", [128, 2, T], BF16,
                                           isOutput=True)
        tap_kT = nc.declare_dram_parameter("tap_kT", [128, 2, T], BF16,
                                           isOutput=True)
        tap_v = nc.declare_dram_parameter("tap_v", [128, KC, GH, 66], BF16,
                                          isOutput=True)
        tap_xh = [nc.declare_dram_parameter(f"tap_xh{q}", [128, 2, 1024], BF16,
                                            isOutput=True) for q in (0, 1)]

    with tile.TileContext(nc) as tc:
        with (
            tc.tile_pool(name="persist", bufs=1) as pp,
            tc.tile_pool(name="psum", bufs=4, space="PSUM") as psp,
        ):
            # persistent sbuf tensors
            wq_sb = pp.tile([128, DCH, GC], BF16, tag="wq")
            wk_sb = pp.tile([128, DCH, GC], BF16, tag="wk")
            wv_sb = pp.tile([128, DCH, GC], BF16, tag="wv")
            wo_sb = pp.tile([128, 2, D], BF16, tag="wo")
            mask_sb = pp.tile([128, KC], F32, tag="mask")
            bq_sb = pp.tile([128, 2], F32, tag="bq")
            bk_sb = pp.tile([128, 2], F32, tag="bk")
            bv_sb = pp.tile([64, GH], F32, tag="bv")
            qT_sb = pp.tile([128, 2, T], BF16, tag="qT")
            kT_sb = pp.tile([128, 2, T], BF16, tag="kT")
            v_sb = pp.tile([128, KC, GH, 66], BF16, tag="v")
            xh_sb = [pp.tile([128, 2, 1024], BF16, tag=f"xh{q}", name=f"xh{q}")
                     for q in (0, 1)]
            nc.vector.memset(v_sb[:, :, :, 64:65], 1.0)

            xvp_cm = tc.tile_pool(name="xv", bufs=1)
            xvp = xvp_cm.__enter__()
            xv_sb = [xvp.tile([128, T], BF16, tag=f"xv{k}", name=f"xv{k}")
                     for k in range(DCH)]

            # ---- DMAs + q/k projections (k first: scores need kT+qT) ----
            with tc.tile_pool(name="xqk", bufs=1) as xp:
                xq_sb = [xp.tile([128, T], BF16, tag=f"xq{k}", name=f"xq{k}")
                         for k in range(DCH)]
                xk_sb = [xp.tile([128, T], BF16, tag=f"xk{k}", name=f"xk{k}")
                         for k in range(DCH)]
                nc.sync.dma_start(out=wk_sb[:], in_=wk_d[:])
                for k in range(DCH):
                    nc.sync.dma_start(out=xk_sb[k][:], in_=xk_d[k])
                nc.sync.dma_start(out=wq_sb[:], in_=wq_d[:])
                nc.sync.dma_start(out=bq_sb[:], in_=bq_d[:])
                nc.sync.dma_start(out=bk_sb[:], in_=bk_d[:])
                for k in range(DCH):
                    nc.sync.dma_start(out=xq_sb[k][:], in_=xq_d[k])
                nc.sync.dma_start(out=mask_sb[:], in_=mask_d[:])
                nc.sync.dma_start(out=wv_sb[:], in_=wv_d[:])
                for k in range(DCH):
                    nc.sync.dma_start(out=xv_sb[k][:], in_=xv_d[k])
                nc.sync.dma_start(out=bv_sb[:], in_=bv_d[:])
                nc.sync.dma_start(out=wo_sb[:], in_=wo_d[:])

                for w_sb, x_sb, dst, b_sb in (
                    (wk_sb, xk_sb, kT_sb, bk_sb),
                    (wq_sb, xq_sb, qT_sb, bq_sb),
                ):
                    for m in range(2):
                        pst = [psp.tile([128, 1024], F32, tag="ps", name="ps")
                               for _ in range(2)]
                        for k in range(DCH):
                            for th in range(2):
                                for n in range(2):
                                    nc.tensor.matmul(
                                        pst[th][:, n * 512:(n + 1) * 512],
                                        w_sb[:, k, m * 128:(m + 1) * 128],
                                        x_sb[k][:, th * 1024 + n * 512:
                                                th * 1024 + (n + 1) * 512],
                                        start=(k == 0), stop=(k == DCH - 1),
                                        skip_group_check=True,
                                    )
                        for th in range(2):
                            nc.vector.tensor_scalar_add(
                                dst[:, m, th * 1024:(th + 1) * 1024],
                                pst[th][:], b_sb[:, m:m + 1])

            # ---- Phase B: software-pipelined attention units ----
            # unit = (qh, pr): scores+exp of unit u run interleaved with the
            # V-matmuls of unit u-1 (whose exps are a full unit old -> no PE
            # stalls). v-projection rides inside unit 0; phase C of qh=0
            # rides inside the last unit; qh=1's phase C is the tail.
            bc_pools = (
                tc.tile_pool(name="expp", bufs=40),
                tc.tile_pool(name="outp", bufs=4),
                tc.tile_pool(name="normp", bufs=2),
            )
            exp_pool = bc_pools[0].__enter__()
            out_pool = bc_pools[1].__enter__()
            norm_pool = bc_pools[2].__enter__()

            def emit_vproj(tcn):
                ps = psp.tile([128, GH, 64], F32, tag="ps", name="vps")
                for k in range(DCH):
                    nc.tensor.matmul(
                        ps[:],
                        xv_sb[k][:, tcn * 128:(tcn + 1) * 128],
                        wv_sb[:, k, :],
                        start=(k == 0), stop=(k == DCH - 1),
                        skip_group_check=True,
                    )
                nc.vector.tensor_copy(v_sb[:, tcn, :, 0:64], ps[:])

            def emit_v(prev, kc, hhs=(0, 1)):
                qh, pr, o2, exs = prev
                for hh in hhs:
                    h = 2 * pr + hh
                    for n in range(2):
                        nc.tensor.matmul(
                            o2[hh][:, n * 512:(n + 1) * 512],
                            v_sb[:, kc, h, 0:65],
                            exs[kc][hh][:, n * 512:(n + 1) * 512],
                            start=(kc == 0), stop=(kc == KC - 1),
                            skip_group_check=True,
                        )

            def emit_norm(prev, half=None):
                qh, pr, o2, exs = prev
                sl = slice(0, 1024) if half is None else \
                    slice(half * 512, (half + 1) * 512)
                w = sl.stop - sl.start
                for hh in range(2):
                    rr = norm_pool.tile([1, 2, 1024], F32, tag="rr", name="rr")
                    nc.vector.tensor_copy(rr[:, 0, :w], o2[hh][64:65, sl])
                    nc.vector.reciprocal_approx_fast(rr[:, 1, :w], rr[:, 0, :w])
                    rb = norm_pool.tile([64, 1024], F32, tag="rb", name="rb")
                    nc.gpsimd.partition_broadcast(rb[:, :w], rr[:, 1, :w])
                    if hh == 0:
                        nc.vector.tensor_mul(
                            xh_sb[qh][0:64, pr, sl], o2[hh][0:64, sl],
                            rb[:, :w])
                        if with_bv:
                            nc.vector.tensor_scalar_add(
                                xh_sb[qh][0:64, pr, sl],
                                xh_sb[qh][0:64, pr, sl],
                                bv_sb[:, 2 * pr:2 * pr + 1])
                    else:
                        tmp = norm_pool.tile([64, 1024], BF16, tag="tmp",
                                             name="tmp")
                        nc.vector.tensor_mul(tmp[:, :w], o2[hh][0:64, sl],
                                             rb[:, :w])
                        if with_bv:
                            nc.vector.tensor_scalar_add(
                                tmp[:, :w], tmp[:, :w],
                                bv_sb[:, 2 * pr + 1:2 * pr + 2])
                        nc.sync.dma_start(
                            out=xh_sb[qh][64:128, pr, sl], in_=tmp[:, :w])

            def emit_outproj(qh, tr, tail=False):
                tcn = qh * 8 + tr
                po = psp.tile([128, 1024], F32, tag="ps", name="po")
                for m in range(2):
                    for n in range(2):
                        nc.tensor.matmul(
                            po[:, n * 512:(n + 1) * 512],
                            xh_sb[qh][:, m, tr * 128:(tr + 1) * 128],
                            wo_sb[:, m, n * 512:(n + 1) * 512],
                            start=(m == 0), stop=(m == 1),
                            skip_group_check=True,
                        )
                ot = out_pool.tile([128, 1024], F32, tag="ot")
                # in the tail ScalarE is idle: alternate copies across engines
                if tail and tr % 2 == 0:
                    nc.scalar.copy(ot[:], po[:])
                else:
                    nc.vector.tensor_copy(ot[:], po[:])
                nc.sync.dma_start(
                    out=out_d[tcn * 128:(tcn + 1) * 128, :], in_=ot[:])

            units = [(0, 0), (0, 1), (1, 0), (1, 1)]
            prev = None
            for ui, (qh, pr) in enumerate(units):
                q0 = qh * 1024
                o2 = [psp.tile([65, 1024], F32, tag="ps", name="o2")
                      for _ in range(2)]
                exs = []
                for kc in range(KC):
                    # interleave the previous unit's (full-array) V matmuls
                    # between this unit's (half-array) score matmuls: same
                    # total PE cycles, but serial -> higher array duty, which
                    # keeps the HAM clock at 2.4 GHz during the ScalarE-bound
                    # attention phase.
                    se = [psp.tile([128, 1024], F32, tag="ps", name="se")
                          for _ in range(2)]
                    if prev is not None:
                        emit_v(prev, kc)
                    for hh in range(2):
                        pb = 64 * hh
                        for n in range(2):
                            nc.tensor.matmul(
                                se[hh][:, n * 512:(n + 1) * 512],
                                kT_sb[pb:pb + 64, pr,
                                      kc * 128:(kc + 1) * 128],
                                qT_sb[pb:pb + 64, pr,
                                      q0 + n * 512:q0 + (n + 1) * 512],
                                start=True, stop=True,
                            )
                    if ui == 0 and kc >= 8:
                        emit_vproj(2 * (kc - 8))
                        emit_vproj(2 * (kc - 8) + 1)
                    ex = [exp_pool.tile([128, 1024], BF16, tag="ex", name="ex")
                          for _ in range(2)]
                    for hh in range(2):
                        nc.scalar.activation(
                            ex[hh][:], se[hh][:],
                            mybir.ActivationFunctionType.Exp,
                            bias=mask_sb[:, kc:kc + 1],
                            scale=float(DK) ** -0.5,
                        )
                    exs.append(ex)
                    if ui == 3 and kc % 2 == 1:
                        emit_outproj(0, kc // 2)
                if prev is not None:
                    emit_norm(prev)
                prev = (qh, pr, o2, exs)

            # tail: the last unit's V accumulation, split in q-halves so the
            # first half's norm (DVE/GpSimd) overlaps the second half's V
            # matmuls (PE); phase-C chunks follow as soon as each norm lands.
            for half in range(2):
                n0 = half * 512
                for kc in range(KC):
                    for hh in range(2):
                        nc.tensor.matmul(
                            prev[2][hh][:, n0:n0 + 512],
                            v_sb[:, kc, 2 * prev[1] + hh, 0:65],
                            prev[3][kc][hh][:, n0:n0 + 512],
                            start=(kc == 0), stop=(kc == KC - 1),
                            skip_group_check=True,
                        )
                emit_norm(prev, half=half)
            for tr in range(8):
                emit_outproj(1, tr, tail=True)

            if debug_taps:
                nc.sync.dma_start(out=tap_qT[:], in_=qT_sb[:])
                nc.sync.dma_start(out=tap_kT[:], in_=kT_sb[:])
                nc.sync.dma_start(out=tap_v[:], in_=v_sb[:])
                for q in (0, 1):
                    nc.sync.dma_start(out=tap_xh[q][:], in_=xh_sb[q][:])
            for _p in reversed(bc_pools):
                _p.__exit__(None, None, None)
            xvp_cm.__exit__(None, None, None)

    nc.compile()
    return nc


_CACHE = {}


def _get_program(with_bv: bool):
    if with_bv not in _CACHE:
        _CACHE[with_bv] = build_program(with_bv)
    return _CACHE[with_bv]


def make_in_maps(query, key, value, mask, Wq, bq, Wk, bk, Wv, bv, Wo, bo):
    bf = ml_dtypes.bfloat16
    # transposed bf16 activations are shared by the 4 cores of each batch
    xt = {}
    for nm, x in (("xq", query), ("xk", key), ("xv", value)):
        for b in range(B):
            xt[nm, b] = np.ascontiguousarray(
                x[b].T.reshape(DCH, 128, T)).astype(bf)
    in_maps = []
    for c in range(NCORES):
        b, g = c // 4, c % 4
        cols = slice(GC * g, GC * (g + 1))
        m = {}
        for nm in ("xq", "xk", "xv"):
            m[nm] = xt[nm, b]
        for nm, W in (("wq", Wq), ("wk", Wk), ("wv", Wv)):
            m[nm] = np.ascontiguousarray(
                W[cols, :].T.reshape(DCH, 128, GC).transpose(1, 0, 2)
            ).astype(bf)
        m["wo"] = np.ascontiguousarray(
            Wo[:, cols].T.reshape(2, 128, D).transpose(1, 0, 2)).astype(bf)
        mb = np.where(mask[b, 0] != 0, 0.0, MASK_NEG).astype(np.float32)
        m["maskb"] = np.ascontiguousarray(mb.reshape(KC, 128).T)
        m["bq"] = np.ascontiguousarray(
            bq[cols].reshape(2, 128).T.astype(np.float32))
        m["bk"] = np.ascontiguousarray(
            bk[cols].reshape(2, 128).T.astype(np.float32))
        m["bv"] = np.ascontiguousarray(
            bv[cols].reshape(GH, 64).T.astype(np.float32))
        in_maps.append(m)
    return in_maps


def kernel(query, key, value, mask, Wq, bq, Wk, bk, Wv, bv, Wo, bo,
           _trace=False):
    query, key, value = (np.asarray(a, np.float32) for a in (query, key, value))
    mask = np.asarray(mask)
    with_bv = bool(np.any(np.asarray(bv)))
    nc = _get_program(with_bv)
    in_maps = make_in_maps(query, key, value, mask, Wq, bq, Wk, bk, Wv, bv,
                           Wo, bo)
    res = run_bass_kernel_spmd(nc, in_maps, list(range(NCORES)), trace=_trace)
    out = np.zeros((B, T, D), np.float32)
    for c in range(NCORES):
        out[c // 4] += res.results[c]["out"]
    out += np.asarray(bo, np.float32)[None, None, :]
    if _trace:
        kernel.last_exec_time_ns = res.exec_time_ns
        kernel.last_results = res
    return out


# revision 31
# speedup vs baseline: 1.0047x; 1.0047x over previous
"""MultiHeadedAttention Trainium2 kernel.

Problem: B=2, T=2048, D=1024, H=16 heads (DK=64), fp32 in/out, padding mask
on keys. out = softmax(mask(QWq (KWk)^T / 8)) @ (VWv) @ Wo^T + biases.

Sharding (8 cores): core c -> batch b = c//4, head group g = c%4 (4 heads,
256 projection columns). Each core computes its heads' attention and a
partial output projection; host sums the 4 partials per batch (+ bo).

Device layout strategy (per core):
  - activations uploaded TRANSPOSED as bf16: xT [1024, 2048] = x.T so the
    D-contraction sits on partitions for every projection matmul.
  - qT/kT computed as [256, 2048] (transposed projections): scores^T[key,q]
    = kT_h.T-chunk (stationary) x qT_h (moving), contraction dk=64.
  - attention units (q-half x head-pair) are software-pipelined: unit u's
    scores/exp interleave with unit u-1's V matmuls (whose exp inputs are a
    full unit old, so the PE queue never blocks on ScalarE); v-projection
    rides in unit 0's back half, qh=0's output projection in the last unit.
  - exp on ScalarE straight out of PSUM with fused scale (1/8) and
    per-partition mask bias (-30000 on padded keys), bf16 out.
  - attn@V in transposed orientation: out2[dk+1, q] with v augmented by a
    ones column -> row 64 = softmax denominator for free.
  - normalize: reciprocal_approx_fast on the denom row, gpsimd
    partition_broadcast, DVE multiply -> x_headsT [256, 2048] bf16 which is
    exactly the stationary operand for the output projection.
  - out_partial[t, :] = x_headsT-chunk (stationary) x WoT (moving), fp32.
"""

import numpy as np
import ml_dtypes

import concourse.bass as bass
import concourse.bacc as bacc
import concourse.tile as tile
from concourse import mybir
from concourse.bass_utils import run_bass_kernel_spmd

B, T, D, H = 2, 2048, 1024, 16
DK = D // H  # 64
GH = 4       # heads per core
GC = GH * DK  # 256 proj columns per core
NCORES = 8
KC = T // 128   # 16 key chunks
DCH = D // 128  # 8 contraction chunks
F32 = mybir.dt.float32
BF16 = mybir.dt.bfloat16

MASK_NEG = -30000.0


def build_program(with_bv: bool, debug_taps: bool = False):
    nc = bacc.Bacc("TRN2")

    # ---- DRAM parameters (per-core shapes) ----
    xq_d = nc.declare_dram_parameter("xq", [DCH, 128, T], BF16, isOutput=False)
    xk_d = nc.declare_dram_parameter("xk", [DCH, 128, T], BF16, isOutput=False)
    xv_d = nc.declare_dram_parameter("xv", [DCH, 128, T], BF16, isOutput=False)
    wq_d = nc.declare_dram_parameter("wq", [128, DCH, GC], BF16, isOutput=False)
    wk_d = nc.declare_dram_parameter("wk", [128, DCH, GC], BF16, isOutput=False)
    wv_d = nc.declare_dram_parameter("wv", [128, DCH, GC], BF16, isOutput=False)
    wo_d = nc.declare_dram_parameter("wo", [128, 2, D], BF16, isOutput=False)
    mask_d = nc.declare_dram_parameter("maskb", [128, KC], F32, isOutput=False)
    bq_d = nc.declare_dram_parameter("bq", [128, 2], F32, isOutput=False)
    bk_d = nc.declare_dram_parameter("bk", [128, 2], F32, isOutput=False)
    bv_d = nc.declare_dram_parameter("bv", [64, GH], F32, isOutput=False)
    out_d = nc.declare_dram_parameter("out", [T, D], F32, isOutput=True)
    if debug_taps:
        tap_qT = nc.declare_dram_parameter("tap_qT", [128, 2, T], BF16,
                                           isOutput=True)
        tap_kT = nc.declare_dram_parameter("tap_kT", [128, 2, T], BF16,
                                           isOutput=True)
        tap_v = nc.declare_dram_parameter("tap_v", [128, KC, GH, 66], BF16,
                                          isOutput=True)
        tap_xh = [nc.declare_dram_parameter(f"tap_xh{q}", [128, 2, 1024], BF16,
                                            isOutput=True) for q in (0, 1)]

    with tile.TileContext(nc) as tc:
        with (
            tc.tile_pool(name="persist", bufs=1) as pp,
            tc.tile_pool(name="psum", bufs=4, space="PSUM") as psp,
        ):
            # persistent sbuf tensors
            wq_sb = pp.tile([128, DCH, GC], BF16, tag="wq")
            wk_sb = pp.tile([128, DCH, GC], BF16, tag="wk")
            wv_sb = pp.tile([128, DCH, GC], BF16, tag="wv")
            wo_sb = pp.tile([128, 2, D], BF16, tag="wo")
            mask_sb = pp.tile([128, KC], F32, tag="mask")
            bq_sb = pp.tile([128, 2], F32, tag="bq")
            bk_sb = pp.tile([128, 2], F32, tag="bk")
            bv_sb = pp.tile([64, GH], F32, tag="bv")
            qT_sb = pp.tile([128, 2, T], BF16, tag="qT")
            kT_sb = pp.tile([128, 2, T], BF16, tag="kT")
            v_sb = pp.tile([128, KC, GH, 66], BF16, tag="v")
            xh_sb = [pp.tile([128, 2, 1024], BF16, tag=f"xh{q}", name=f"xh{q}")
                     for q in (0, 1)]
            nc.vector.memset(v_sb[:, :, :, 64:65], 1.0)

            xvp_cm = tc.tile_pool(name="xv", bufs=1)
            xvp = xvp_cm.__enter__()
            xv_sb = [xvp.tile([128, T], BF16, tag=f"xv{k}", name=f"xv{k}")
                     for k in range(DCH)]

            # ---- DMAs + q/k projections (k first: scores need kT+qT) ----
            with tc.tile_pool(name="xqk", bufs=1) as xp:
                xq_sb = [xp.tile([128, T], BF16, tag=f"xq{k}", name=f"xq{k}")
                         for k in range(DCH)]
                xk_sb = [xp.tile([128, T], BF16, tag=f"xk{k}", name=f"xk{k}")
                         for k in range(DCH)]
                nc.sync.dma_start(out=wk_sb[:], in_=wk_d[:])
                for k in range(DCH):
                    nc.sync.dma_start(out=xk_sb[k][:], in_=xk_d[k])
                nc.sync.dma_start(out=wq_sb[:], in_=wq_d[:])
                nc.sync.dma_start(out=bq_sb[:], in_=bq_d[:])
                nc.sync.dma_start(out=bk_sb[:], in_=bk_d[:])
                for k in range(DCH):
                    nc.sync.dma_start(out=xq_sb[k][:], in_=xq_d[k])
                nc.sync.dma_start(out=mask_sb[:], in_=mask_d[:])
                nc.sync.dma_start(out=wv_sb[:], in_=wv_d[:])
                for k in range(DCH):
                    nc.sync.dma_start(out=xv_sb[k][:], in_=xv_d[k])
                nc.sync.dma_start(out=bv_sb[:], in_=bv_d[:])
                nc.sync.dma_start(out=wo_sb[:], in_=wo_d[:])

                for w_sb, x_sb, dst, b_sb in (
                    (wk_sb, xk_sb, kT_sb, bk_sb),
                    (wq_sb, xq_sb, qT_sb, bq_sb),
                ):
                    for m in range(2):
                        pst = [psp.tile([128, 1024], F32, tag="ps", name="ps")
                               for _ in range(2)]
                        for k in range(DCH):
                            for th in range(2):
                                for n in range(2):
                                    nc.tensor.matmul(
                                        pst[th][:, n * 512:(n + 1) * 512],
                                        w_sb[:, k, m * 128:(m + 1) * 128],
                                        x_sb[k][:, th * 1024 + n * 512:
                                                th * 1024 + (n + 1) * 512],
                                        start=(k == 0), stop=(k == DCH - 1),
                                        skip_group_check=True,
                                    )
                        for th in range(2):
                            nc.vector.tensor_scalar_add(
                                dst[:, m, th * 1024:(th + 1) * 1024],
                                pst[th][:], b_sb[:, m:m + 1])

            # ---- Phase B: software-pipelined attention units ----
            # unit = (qh, pr): scores+exp of unit u run interleaved with the
            # V-matmuls of unit u-1 (whose exps are a full unit old -> no PE
            # stalls). v-projection rides inside unit 0; phase C of qh=0
            # rides inside the last unit; qh=1's phase C is the tail.
            bc_pools = (
                tc.tile_pool(name="expp", bufs=40),
                tc.tile_pool(name="outp", bufs=4),
                tc.tile_pool(name="normp", bufs=2),
            )
            exp_pool = bc_pools[0].__enter__()
            out_pool = bc_pools[1].__enter__()
            norm_pool = bc_pools[2].__enter__()

            def emit_vproj(tcn):
                ps = psp.tile([128, GH, 64], F32, tag="ps", name="vps")
                for k in range(DCH):
                    nc.tensor.matmul(
                        ps[:],
                        xv_sb[k][:, tcn * 128:(tcn + 1) * 128],
                        wv_sb[:, k, :],
                        start=(k == 0), stop=(k == DCH - 1),
                        skip_group_check=True,
                    )
                nc.vector.tensor_copy(v_sb[:, tcn, :, 0:64], ps[:])

            def emit_v(prev, kc, hhs=(0, 1)):
                qh, pr, o2, exs = prev
                for hh in hhs:
                    h = 2 * pr + hh
                    for n in range(2):
                        nc.tensor.matmul(
                            o2[hh][:, n * 512:(n + 1) * 512],
                            v_sb[:, kc, h, 0:65],
                            exs[kc][hh][:, n * 512:(n + 1) * 512],
                            start=(kc == 0), stop=(kc == KC - 1),
                            skip_group_check=True,
                        )

            def emit_norm(prev, half=None):
                qh, pr, o2, exs = prev
                sl = slice(0, 1024) if half is None else \
                    slice(half * 512, (half + 1) * 512)
                w = sl.stop - sl.start
                for hh in range(2):
                    rr = norm_pool.tile([1, 2, 1024], F32, tag="rr", name="rr")
                    nc.vector.tensor_copy(rr[:, 0, :w], o2[hh][64:65, sl])
                    nc.vector.reciprocal_approx_fast(rr[:, 1, :w], rr[:, 0, :w])
                    rb = norm_pool.tile([64, 1024], F32, tag="rb", name="rb")
                    nc.gpsimd.partition_broadcast(rb[:, :w], rr[:, 1, :w])
                    if hh == 0:
                        nc.vector.tensor_mul(
                            xh_sb[qh][0:64, pr, sl], o2[hh][0:64, sl],
                            rb[:, :w])
                        if with_bv:
                            nc.vector.tensor_scalar_add(
                                xh_sb[qh][0:64, pr, sl],
                                xh_sb[qh][0:64, pr, sl],
                                bv_sb[:, 2 * pr:2 * pr + 1])
                    else:
                        tmp = norm_pool.tile([64, 1024], BF16, tag="tmp",
                                             name="tmp")
                        nc.vector.tensor_mul(tmp[:, :w], o2[hh][0:64, sl],
                                             rb[:, :w])
                        if with_bv:
                            nc.vector.tensor_scalar_add(
                                tmp[:, :w], tmp[:, :w],
                                bv_sb[:, 2 * pr + 1:2 * pr + 2])
                        nc.sync.dma_start(
                            out=xh_sb[qh][64:128, pr, sl], in_=tmp[:, :w])

            def emit_outproj(qh, tr, tail=False):
                tcn = qh * 8 + tr
                po = psp.tile([128, 1024], F32, tag="ps", name="po")
                for m in range(2):
                    for n in range(2):
                        nc.tensor.matmul(
                            po[:, n * 512:(n + 1) * 512],
                            xh_sb[qh][:, m, tr * 128:(tr + 1) * 128],
                            wo_sb[:, m, n * 512:(n + 1) * 512],
                            start=(m == 0), stop=(m == 1),
                            skip_group_check=True,
                        )
                ot = out_pool.tile([128, 1024], F32, tag="ot")
                # in the tail ScalarE is idle: alternate copies across engines
                if tail and tr % 2 == 0:
                    nc.scalar.copy(ot[:], po[:])
                else:
                    nc.vector.tensor_copy(ot[:], po[:])
                nc.sync.dma_start(
                    out=out_d[tcn * 128:(tcn + 1) * 128, :], in_=ot[:])

            units = [(0, 0), (0, 1), (1, 0), (1, 1)]
            prev = None
            for ui, (qh, pr) in enumerate(units):
                q0 = qh * 1024
                o2 = [psp.tile([65, 1024], F32, tag="ps", name="o2")
                      for _ in range(2)]
                exs = []
                for kc in range(KC):
                    # interleave the previous unit's (full-array) V matmuls
                    # between this unit's (half-array) score matmuls: same
                    # total PE cycles, but serial -> higher array duty, which
                    # keeps the HAM clock at 2.4 GHz during the ScalarE-bound
                    # attention phase.
                    se = [psp.tile([128, 1024], F32, tag="ps", name="se")
                          for _ in range(2)]
                    if prev is not None:
                        emit_v(prev, kc)
                    for hh in range(2):
                        pb = 64 * hh
                        for n in range(2):
                            nc.tensor.matmul(
                                se[hh][:, n * 512:(n + 1) * 512],
                                kT_sb[pb:pb + 64, pr,
                                      kc * 128:(kc + 1) * 128],
                                qT_sb[pb:pb + 64, pr,
                                      q0 + n * 512:q0 + (n + 1) * 512],
                                start=True, stop=True,
                            )
                    if ui == 0 and kc >= 8:
                        emit_vproj(2 * (kc - 8))
                        emit_vproj(2 * (kc - 8) + 1)
                    ex = [exp_pool.tile([128, 1024], BF16, tag="ex", name="ex")
                          for _ in range(2)]
                    for hh in range(2):
                        nc.scalar.activation(
                            ex[hh][:], se[hh][:],
                            mybir.ActivationFunctionType.Exp,
                            bias=mask_sb[:, kc:kc + 1],
                            scale=float(DK) ** -0.5,
                        )
                    exs.append(ex)
                    if ui == 3 and kc % 2 == 1:
                        emit_outproj(0, kc // 2)
                if prev is not None:
                    emit_norm(prev)
                prev = (qh, pr, o2, exs)

            # tail: the last unit's V accumulation, split in q-halves so the
            # first half's norm (DVE/GpSimd) overlaps the second half's V
            # matmuls (PE); phase-C chunks follow as soon as each norm lands.
            for half in range(2):
                n0 = half * 512
                for kc in range(KC):
                    for hh in range(2):
                        nc.tensor.matmul(
                            prev[2][hh][:, n0:n0 + 512],
                            v_sb[:, kc, 2 * prev[1] + hh, 0:65],
                            prev[3][kc][hh][:, n0:n0 + 512],
                            start=(kc == 0), stop=(kc == KC - 1),
                            skip_group_check=True,
                        )
                emit_norm(prev, half=half)
            for tr in range(8):
                emit_outproj(1, tr, tail=True)

            if debug_taps:
                nc.sync.dma_start(out=tap_qT[:], in_=qT_sb[:])
                nc.sync.dma_start(out=tap_kT[:], in_=kT_sb[:])
                nc.sync.dma_start(out=tap_v[:], in_=v_sb[:])
                for q in (0, 1):
                    nc.sync.dma_start(out=tap_xh[q][:], in_=xh_sb[q][:])
            for _p in reversed(bc_pools):
                _p.__exit__(None, None, None)
            xvp_cm.__exit__(None, None, None)

    nc.compile()
    return nc


_CACHE = {}


def _get_program(with_bv: bool):
    if with_bv not in _CACHE:
        _CACHE[with_bv] = build_program(with_bv)
    return _CACHE[with_bv]


def make_in_maps(query, key, value, mask, Wq, bq, Wk, bk, Wv, bv, Wo, bo):
    bf = ml_dtypes.bfloat16
    # transposed bf16 activations are shared by the 4 cores of each batch
    xt = {}
    for nm, x in (("xq", query), ("xk", key), ("xv", value)):
        for b in range(B):
            xt[nm, b] = np.ascontiguousarray(
                x[b].T.reshape(DCH, 128, T)).astype(bf)
    in_maps = []
    for c in range(NCORES):
        b, g = c // 4, c % 4
        cols = slice(GC * g, GC * (g + 1))
        m = {}
        for nm in ("xq", "xk", "xv"):
            m[nm] = xt[nm, b]
        for nm, W in (("wq", Wq), ("wk", Wk), ("wv", Wv)):
            m[nm] = np.ascontiguousarray(
                W[cols, :].T.reshape(DCH, 128, GC).transpose(1, 0, 2)
            ).astype(bf)
        m["wo"] = np.ascontiguousarray(
            Wo[:, cols].T.reshape(2, 128, D).transpose(1, 0, 2)).astype(bf)
        mb = np.where(mask[b, 0] != 0, 0.0, MASK_NEG).astype(np.float32)
        m["maskb"] = np.ascontiguousarray(mb.reshape(KC, 128).T)
        m["bq"] = np.ascontiguousarray(
            bq[cols].reshape(2, 128).T.astype(np.float32))
        m["bk"] = np.ascontiguousarray(
            bk[cols].reshape(2, 128).T.astype(np.float32))
        m["bv"] = np.ascontiguousarray(
            bv[cols].reshape(GH, 64).T.astype(np.float32))
        in_maps.append(m)
    return in_maps


def kernel(query, key, value, mask, Wq, bq, Wk, bk, Wv, bv, Wo, bo,
           _trace=False):
    query, key, value = (np.asarray(a, np.float32) for a in (query, key, value))
    mask = np.asarray(mask)
    with_bv = bool(np.any(np.asarray(bv)))
    nc = _get_program(with_bv)
    in_maps = make_in_maps(query, key, value, mask, Wq, bq, Wk, bk, Wv, bv,
                           Wo, bo)
    res = run_bass_kernel_spmd(nc, in_maps, list(range(NCORES)), trace=_trace)
    out = np.zeros((B, T, D), np.float32)
    for c in range(NCORES):
        out[c // 4] += res.results[c]["out"]
    out += np.asarray(bo, np.float32)[None, None, :]
    if _trace:
        kernel.last_exec_time_ns = res.exec_time_ns
        kernel.last_results = res
    return out
